# revision 1
# baseline (speedup 1.0000x reference)
"""CRF-RNN (dense Gaussian CRF mean-field) Trainium2 kernel, 8 NeuronCores.

Strategy
--------
N = 8*32*32 = 8192 voxels, L = 21 labels, 5 mean-field iterations.
- Column sharding: core r owns z-slice r (1024 voxels = its output columns).
- Bilateral kernel K_b: all pairwise exponents -d^2/2 lie in [-0.206, 0]
  (ALPHA=160, BETA=3 make distances tiny), so exp is replaced by the
  real-rooted minimax quadratic p(x) = PC*(x+PA)^2 (max rel err 1.3e-3,
  which cancels almost entirely under the host-computed normalization).
  p factors as a 36-dim polynomial feature map (S^2 with
  S = alpha_i + beta_c + f_i.f_c), so the PE emits the finished kernel
  value and each chunk needs only a PSUM->SBUF casting copy to fp8,
  split round-robin across ACT / DVE to lift the old ACT bottleneck.
- Normalizer 1/sum_i K_b[i,c] is computed on host with the same quadratic
  and shipped pre-broadcast as recipb [L, NYX].
- Spatial kernel is exactly separable (grid tensor product):
  q@K_s[:, cols_r] = (sum_z Gz[z,r] q[:,z]) @ (Gy x Gx); norm_s folded in.
- Iterations are pipelined by 512-column halves (epilogue / fused mixing /
  softmax per half); q is exchanged as fp8 via one AllGather per iteration,
  and the bilateral consumes gathered chunks in arrival-friendly order.
- Softmax runs voxel-major (labels on the free dim) - no cross-partition ops.
"""

import numpy as np

ALPHA, BETA, GAMMA = 160.0, 3.0, 3.0
NUM_ITER = 5
L, D, H, W = 21, 8, 32, 32
NC = 8
NYX = H * W            # 1024
N = D * NYX            # 8192
NT = NYX // 128        # 8 chunks per slice
FB = NT * L            # 168  free width of one q block
NTILE = N // 128       # 64 row tiles of the bilateral kernel

PA = 2.105             # quadratic exp fit: exp(x) ~= PC*(x+PA)^2 on [-0.21,0]
PC = 0.22538087
NF = 36                # polynomial feature-map width
NFH = NF // 2          # features per DoubleRow k-subtile

_CACHE = {}


def _chunk_engines():
    """Interleaved ACT/DVE schedule for the 64 kernel chunks, weighted by
    estimated per-chunk service time (Pool cannot read PSUM on trn2)."""
    rates = {"A": 1.0 / 1.40, "D": 1.0 / 1.19}
    tot = sum(rates.values())
    shares = {k: v / tot for k, v in rates.items()}
    used = {k: 0 for k in rates}
    seq = []
    for i in range(NTILE):
        pick = max(rates, key=lambda k: shares[k] * (i + 1) - used[k])
        used[pick] += 1
        seq.append(pick)
    return seq


def _build_nc():
    import concourse.bass as bass
    import concourse.bacc as bacc
    import concourse.mybir as mybir
    import concourse.tile as tile
    import concourse.tile_utils as tile_utils

    # cayman has 208KB/partition usable; the default cap is stale at 192KB
    try:
        tile_utils.max_sbuf_usage = 204 * 1024
    except Exception:
        pass

    f32 = mybir.dt.float32
    f16 = mybir.dt.float16
    f8 = mybir.dt.float8e4
    AF = mybir.ActivationFunctionType
    OP = mybir.AluOpType
    DR = mybir.MatmulPerfMode.DoubleRow

    nc = bacc.Bacc(None, target_bir_lowering=False, num_devices=NC)

    # ---- DRAM I/O (fp16 features; global voxel order everywhere) ----
    featr_d = nc.declare_dram_parameter("featr", [NF, N], f16, isOutput=False)
    featc_d = nc.declare_dram_parameter("featc", [NF, NYX], f16, isOutput=False)
    kyx_d = nc.declare_dram_parameter("kyx", [128, NT * NYX], f16, isOutput=False)
    unary_d = nc.declare_dram_parameter("unaryt", [128, NC * FB], f32, isOutput=False)
    unown_d = nc.declare_dram_parameter("unown", [128, FB], f32, isOutput=False)
    zco_d = nc.declare_dram_parameter("zcoef", [128, NC], f32, isOutput=False)
    zcoo_d = nc.declare_dram_parameter("zcoo", [128, 1], f32, isOutput=False)
    zcor_d = nc.declare_dram_parameter("zcor", [128, NC], f32, isOutput=False)
    w12t_d = nc.declare_dram_parameter("w12t", [53, L], f32, isOutput=False)
    recip_d = nc.declare_dram_parameter("recipb", [L, NYX], f32, isOutput=False)
    out_d = nc.declare_dram_parameter("out", [128, FB], f32, isOutput=True)

    with tile.TileContext(nc) as tc:
        with (
            tc.tile_pool(name="persist", bufs=1) as pp,
            tc.tile_pool(name="stream", bufs=2) as sp,
            tc.tile_pool(name="epi", bufs=12) as ep,
            tc.tile_pool(name="work", bufs=8) as wp,
            tc.tile_pool(name="dram", bufs=2, space="DRAM") as dp,
            tc.tile_pool(name="ps_bil", bufs=1, space="PSUM") as ps_bil,
        ):
            # ---------------- persistent SBUF ----------------
            sb_kb = pp.tile([128, NTILE * NYX], f8, tag="kb")        # 64KB/p
            sb_kyx = pp.tile([128, NT * NYX], f16, tag="kyx")        # 16KB/p
            sb_unary = pp.tile([128, NC * FB], f32, tag="unary")     # 5.25KB/p
            sb_unown = pp.tile([128, FB], f32, tag="unown")
            sb_s1 = pp.tile([128, NC * FB], f8, tag="s1")            # 1.3KB/p
            sb_slots = [pp.tile([128, NC * FB], f8, tag=f"slots{j % 2}",
                                name=f"sb_slots{j}")
                        for j in range(NUM_ITER - 1)]                # ping-pong
            sb_featc = pp.tile([NF, NYX], f16, tag="featc")
            sb_zco = pp.tile([128, NC], f32, tag="zco")
            sb_zcoo = pp.tile([128, 1], f32, tag="zcoo")
            sb_zcor = pp.tile([128, NC], f32, tag="zcor")
            sb_w12t = pp.tile([53, L], f32, tag="w12t")
            sb_recipb = pp.tile([L, NYX], f32, tag="recipb")
            sb_exp1 = pp.tile([128, NC * FB], f16, tag="exp1")       # iter-1 exp
            sb_red1 = pp.tile([128, NC * NT], f32, tag="red1")
            sb_rcp1 = pp.tile([128, NC * NT], f32, tag="rcp1")
            sb_out = pp.tile([128, FB], f32, tag="outt")

            # featc gates the very first build matmul - it goes alone so
            # nothing sits ahead of it on the sync queue
            nc.sync.dma_start(sb_featc[:, :], featc_d[:, :])

            # warm up the collective path early (overlaps the K_b build)
            wu_in = dp.tile([128, 8], f16, tag="wuin")
            wu_out = dp.tile([128 * NC, 8], f16, tag="wuout")
            wu_sb = pp.tile([128, 8], f16, tag="wusb")
            nc.vector.memset(wu_sb[:, :], 0.0)

            # ---------------- K_b build ----------------
            # one PE matmul per chunk emits the finished kernel value
            # PC*(PA - d^2/2)^2 via the 36-dim polynomial feature map; the
            # only post-op is a casting PSUM->SBUF copy on ACT / DVE / Pool.
            engs = _chunk_engines()
            kb_v = sb_kb[:, :].rearrange("p (n c) -> p n c", c=NYX)      # [128, 64, 1024]
            with tc.tile_pool(name="ps_g", bufs=3, space="PSUM") as ps_g:
                for mc in range(NT):  # macro chunks of 8 tiles
                    fr = sp.tile([NF, NYX], f16, tag="fr")
                    nc.sync.dma_start(fr[:, :], featr_d[:, mc * NYX:(mc + 1) * NYX])
                    if mc == 0:
                        # the rest of the inputs queue behind the first
                        # feature stream; none is needed before ~40us in
                        nc.sync.dma_start(sb_unary[:, :], unary_d[:, :])
                        nc.sync.dma_start(sb_zco[:, :], zco_d[:, :])
                        nc.sync.dma_start(sb_zcoo[:, :], zcoo_d[:, :])
                        nc.sync.dma_start(sb_zcor[:, :], zcor_d[:, :])
                        nc.sync.dma_start(sb_w12t[:, :], w12t_d[:, :])
                        nc.sync.dma_start(sb_recipb[:, :], recip_d[:, :])
                        nc.sync.dma_start(sb_unown[:, :], unown_d[:, :])
                        nc.sync.dma_start(wu_in[:, :], wu_sb[:, :])
                        nc.gpsimd.collective_compute(
                            "AllGather", mybir.AluOpType.bypass,
                            replica_groups=[list(range(NC))],
                            ins=[wu_in.opt()], outs=[wu_out.opt()],
                        )
                    for tl in range(NT):
                        dt = mc * NT + tl
                        g = ps_g.tile([128, NYX], f32, tag="g")
                        for h in range(2):
                            nc.tensor.matmul(
                                g[:, h * 512:(h + 1) * 512],
                                fr[:, tl * 128:(tl + 1) * 128],
                                sb_featc[:, h * 512:(h + 1) * 512],
                                start=True, stop=True,
                            )
                        e = engs[dt]
                        if e == "A":
                            nc.scalar.copy(kb_v[:, dt, :], g[:, :])
                        elif e == "D":
                            nc.vector.tensor_copy(kb_v[:, dt, :], g[:, :])
                        else:
                            nc.gpsimd.tensor_copy(kb_v[:, dt, :], g[:, :])

            # kyx is only needed by the iter-0 spatial filter; issuing it
            # here keeps the 2MB transfer out of the build's DMA path
            nc.sync.dma_start(sb_kyx[:, :], kyx_d[:, :])

            # ---------------- iter-1 softmax for all 8 slices ----------------
            nc.scalar.activation(sb_exp1[:, :], sb_unary[:, :], AF.Exp)
            un_v = sb_exp1[:, :].rearrange("p (g l) -> p g l", l=L)      # [128, 64, 21]
            nc.vector.tensor_reduce(sb_red1[:, :], un_v, mybir.AxisListType.X, OP.add)
            nc.vector.reciprocal(sb_rcp1[:, :], sb_red1[:, :])
            s1_v = sb_s1[:, :].rearrange("p (g l) -> p g l", l=L)
            nc.vector.tensor_tensor(
                s1_v, un_v,
                sb_rcp1[:, :].broadcast_to([128, NC * NT, L]),
                OP.mult,
            )


            with tc.tile_pool(name="ps_rest", bufs=1, space="PSUM") as ps_r:
                # ================= iterations =================
                slots_j = sb_s1[:, :].rearrange(
                    "p (j s g l) -> p j s g l", s=2, g=4, l=L)
                slots_dt = sb_s1[:, :].rearrange(
                    "p (d t l) -> p d t l", d=NC, l=L)
                kb_j = sb_kb[:, :].rearrange(
                    "p (j s g c) -> p j s g c", s=2, g=4, c=NYX)

                for it in range(NUM_ITER):
                    last = it == NUM_ITER - 1
                    cur_bil = ps_bil.tile([128, NYX], f32, tag="bil")
                    # ---- bilateral: 4x column-tiled fp8. iter 0 consumes
                    # chunks in build-drain order (dt ascending) so it hides
                    # under the drain tail; later iterations go s-major so
                    # the AG-A half is consumed before AG-B arrives ----
                    if it == 0:
                        order = [(dt // 8, (dt % 8) // 4)
                                 for dt in range(0, NTILE, 4)]
                    else:
                        order = [(m, s) for s in range(2) for m in range(NC)]
                    for idx, (m, s) in enumerate(order):
                        for hb in range(2):
                            for g in range(4):
                                nc.tensor.matmul(
                                    cur_bil[32 * g:32 * g + L,
                                            hb * 512:(hb + 1) * 512],
                                    slots_j[:, m, s, g, :],
                                    kb_j[:, m, s, g,
                                         hb * 512:(hb + 1) * 512],
                                    start=(idx == 0),
                                    stop=(idx == len(order) - 1),
                                    skip_group_check=True,
                                    tile_position=(0, 32 * g),
                                )

                    # ---- z-mix per t-half (DVE) ----
                    kyx_v = sb_kyx[:, :].rearrange("p (k c) -> p k c", c=NYX)
                    zmix = sb_zco if it == 0 else sb_zcor
                    brt_h = []
                    for sb in range(2):
                        b = wp.tile([128, 4 * L], f16, tag=f"brt{sb}")
                        nc.vector.tensor_scalar_mul(
                            b[:, :], slots_dt[:, 0, sb * 4:(sb + 1) * 4, :],
                            zmix[:, 0:1])
                        for d in range(1, NC):
                            nc.vector.scalar_tensor_tensor(
                                b[:, :],
                                slots_dt[:, d, sb * 4:(sb + 1) * 4, :],
                                zmix[:, d:d + 1], b[:, :], OP.mult, OP.add)
                        brt_h.append(b[:, :].rearrange("p (t l) -> p t l", l=L))

                    if it == 0:
                        ps_sp = ps_r.tile([L, NYX], f32, tag="spat")
                    else:
                        ps_sp = ps_sp_pending

                    def spat(hb):
                        for k in range(NT):
                            nc.tensor.matmul(
                                ps_sp[:, hb * 512:(hb + 1) * 512],
                                brt_h[k // 4][:, k % 4, :],
                                kyx_v[:, k, hb * 512:(hb + 1) * 512],
                                start=(k == 0 and it == 0),
                                stop=(k == NT - 1),
                                skip_group_check=True,
                            )

                    spat(0)

                    # ---- bilateral epilogue per half: 4-group sum + norm;
                    # mbn lands in rows 0-20 and msn in rows 32-52 of one
                    # [53,512] tile (rows 21-31 are killed by zero weight
                    # rows in w12t), halving the mixing matmul count ----
                    cat_h = [ep.tile([53, 512], f32, tag="epi",
                                     name=f"cat{it}_{hh}") for hh in range(2)]
                    aa = []
                    for hb in range(2):
                        cs = slice(hb * 512, (hb + 1) * 512)
                        a1 = ep.tile([L, 512], f32, tag="epi")
                        nc.scalar.copy(a1[:, :], cur_bil[32:32 + L, cs])
                        a3 = ep.tile([L, 512], f32, tag="epi")
                        nc.scalar.copy(a3[:, :], cur_bil[96:96 + L, cs])
                        aa.append((a1, a3))
                    for hb in range(2):
                        cs = slice(hb * 512, (hb + 1) * 512)
                        a1, a3 = aa[hb]
                        t1 = ep.tile([L, 512], f32, tag="epi")
                        nc.vector.tensor_tensor(
                            t1[:, :], cur_bil[0:L, cs], a1[:, :], OP.add)
                        t2 = ep.tile([L, 512], f32, tag="epi")
                        nc.vector.tensor_tensor(
                            t2[:, :], cur_bil[64:64 + L, cs], a3[:, :], OP.add)
                        u = ep.tile([L, 512], f32, tag="epi")
                        nc.vector.tensor_tensor(
                            u[:, :], t1[:, :], t2[:, :], OP.add)
                        nc.vector.tensor_tensor(
                            cat_h[hb][0:L, :], u[:, :], sb_recipb[:, cs],
                            OP.mult)

                    ps_ct = ps_r.tile([128, FB], f32, tag="curt")

                    def mix(hb):
                        cat = cat_h[hb]
                        for tl in range(4):
                            tg = hb * 4 + tl
                            nc.tensor.matmul(
                                ps_ct[:, tg * L:(tg + 1) * L],
                                cat[:, tl * 128:(tl + 1) * 128],
                                sb_w12t[:, :], start=True, stop=True)

                    FBH = 4 * L                       # 84: half of FB
                    qb_h, bo_h, cc_pend = [], [], []

                    def prestart(hb):
                        for kl in range(4):
                            kg = hb * 4 + kl
                            for h2 in range(2):
                                nc.tensor.matmul(
                                    ps_sp_pending[:, h2 * 512:(h2 + 1) * 512],
                                    bo_h[hb][:, kl, :],
                                    kyx_v[:, kg, h2 * 512:(h2 + 1) * 512],
                                    start=(kg == 0), stop=False,
                                    skip_group_check=True,
                                )

                    def softmax_half(hb):
                        fs = slice(hb * FBH, (hb + 1) * FBH)
                        sm = wp.tile([128, FBH], f32, tag="sum")
                        nc.vector.tensor_tensor(
                            sm[:, :], ps_ct[:, fs], sb_unown[:, fs], OP.add)
                        ex = wp.tile([128, FBH], f32, tag="exp")
                        nc.scalar.activation(ex[:, :], sm[:, :], AF.Exp)
                        ex_v = ex[:, :].rearrange("p (t l) -> p t l", l=L)
                        rd = wp.tile([128, 4], f32, tag="red")
                        nc.vector.tensor_reduce(
                            rd[:, :], ex_v, mybir.AxisListType.X, OP.add)
                        rc = wp.tile([128, 4], f32, tag="rcp")
                        nc.vector.reciprocal(rc[:, :], rd[:, :])
                        if last:
                            nc.vector.tensor_tensor(
                                sb_out[:, fs].rearrange(
                                    "p (t l) -> p t l", l=L),
                                ex_v, rc[:, :].broadcast_to([128, 4, L]),
                                OP.mult)
                            nc.sync.dma_start(out_d[:, fs], sb_out[:, fs])
                        else:
                            qblk = wp.tile([128, FBH], f8, tag=f"qblk{hb}")
                            nc.vector.tensor_tensor(
                                qblk[:, :].rearrange("p (t l) -> p t l", l=L),
                                ex_v, rc[:, :].broadcast_to([128, 4, L]),
                                OP.mult)
                            qb_h.append(qblk)
                            bo = wp.tile([128, FBH], f16, tag=f"brto{hb}")
                            nc.vector.tensor_scalar_mul(
                                bo[:, :], qblk[:, :], sb_zcoo[:, 0:1])
                            bo_h.append(bo[:, :].rearrange(
                                "p (t l) -> p t l", l=L))

                    spat(1)
                    nc.scalar.copy(cat_h[0][32:53, :], ps_sp[:, 0:512])
                    nc.scalar.copy(cat_h[1][32:53, :], ps_sp[:, 512:1024])
                    mix(0)
                    if not last:
                        ps_sp_pending = ps_r.tile([L, NYX], f32, tag="spat",
                                                  name=f"ps_spp{it}")
                    softmax_half(0)
                    mix(1)
                    softmax_half(1)
                    if not last:
                        # ---- exchange: one AllGather for both q halves (a
                        # CC op costs ~5-7us flat, so one beats two) ----
                        cc_in = dp.tile([128, FB], f8, tag="ccin")
                        cc_out = dp.tile([128 * NC, FB], f8, tag="ccout")
                        nc.sync.dma_start(cc_in[:, 0:FBH], qb_h[0][:, :])
                        nc.sync.dma_start(cc_in[:, FBH:FB], qb_h[1][:, :])
                        nc.gpsimd.collective_compute(
                            "AllGather",
                            mybir.AluOpType.bypass,
                            replica_groups=[list(range(NC))],
                            ins=[cc_in.opt()],
                            outs=[cc_out.opt()],
                        )
                        prestart(0)
                        prestart(1)
                        nxt = sb_slots[it]
                        nc.sync.dma_start(
                            nxt[:, :].rearrange("p (d f) -> p d f", d=NC),
                            cc_out[:, :].rearrange("(d p) f -> p d f", p=128),
                        )
                        slots_j = nxt[:, :].rearrange(
                            "p (j s g l) -> p j s g l", s=2, g=4, l=L)
                        slots_dt = nxt[:, :].rearrange(
                            "p (d t l) -> p d t l", d=NC, l=L)
    nc.compile()
    return nc


def _host_prep(image, logits):
    """Per-core input dicts (global voxel order). Returns list of 8 dicts."""
    img = np.asarray(image, dtype=np.float32)[0]      # [3, D, H, W]
    lg = np.asarray(logits, dtype=np.float32)[0]      # [L, D, H, W]

    zz, yy, xx = np.meshgrid(
        np.arange(D), np.arange(H), np.arange(W), indexing="ij")
    pos = np.stack([zz, yy, xx], -1).reshape(N, 3).astype(np.float32)
    rgb = img.reshape(3, N).T
    feat = np.concatenate([pos / ALPHA, rgb / BETA], axis=1).astype(np.float16)
    f = feat.astype(np.float32)                       # [N, 6] fp16-rounded
    sq = np.sum(f * f, axis=1)
    alpha = PA / 2 - sq / 2                           # row shift
    beta = alpha                                      # same formula per column

    # 36-dim feature map: K_b[i,c] = R(i) . C(c) = PC*(alpha_i+beta_c+f_i.f_c)^2
    pairs = [(a, b) for a in range(6) for b in range(a + 1, 6)]

    def mono_row(ff, al):
        cols = [PC * al * al, PC * np.ones_like(al), PC * 2 * al]
        cols += [PC * 2 * al * ff[:, a] for a in range(6)]
        cols += [PC * 2 * ff[:, a] for a in range(6)]
        cols += [PC * ff[:, a] ** 2 for a in range(6)]
        cols += [PC * 2 * ff[:, a] * ff[:, b] for a, b in pairs]
        return np.stack(cols, 0)                      # [36, n]

    def mono_col(ff, be):
        cols = [np.ones_like(be), be * be, be]
        cols += [ff[:, a] for a in range(6)]
        cols += [be * ff[:, a] for a in range(6)]
        cols += [ff[:, a] ** 2 for a in range(6)]
        cols += [ff[:, a] * ff[:, b] for a, b in pairs]
        return np.stack(cols, 0)                      # [36, n]

    featr = np.ascontiguousarray(mono_row(f, alpha).astype(np.float16))
    rsum = featr.astype(np.float32).sum(axis=1)       # [36] for the normalizer

    r1 = np.arange(D, dtype=np.float32)
    Gz = np.exp(-0.5 * ((r1[:, None] - r1[None, :]) / GAMMA) ** 2)
    r2 = np.arange(H, dtype=np.float32)
    Gy = np.exp(-0.5 * ((r2[:, None] - r2[None, :]) / GAMMA) ** 2)
    Kyx = np.kron(Gy, Gy).astype(np.float32)          # H == W so Gy == Gx
    nyx = Kyx.sum(axis=0)
    Kyx_n = (Kyx / nyx[None, :]).astype(np.float16)   # [1024, 1024]
    czsum = Gz.sum(axis=0)

    unary = lg.reshape(L, N)
    # voxel-major: blkT[p, s, t*L + l] = unary[l, s*NYX + t*128 + p]
    blkT = unary.reshape(L, D, NT, 128).transpose(3, 1, 2, 0)  # [128, D, NT, L]
    un = np.ascontiguousarray(blkT.reshape(128, NC * FB))

    kyx_in = np.ascontiguousarray(
        Kyx_n.reshape(NT, 128, NYX).transpose(1, 0, 2).reshape(128, NT * NYX))

    maps = []
    for r in range(NC):
        sl = slice(r * NYX, (r + 1) * NYX)
        featc = np.ascontiguousarray(
            mono_col(f[sl], beta[sl]).astype(np.float16))  # [36, 1024]
        # normalizer: sum_i K_b[i,c] = (sum_i R(i)) . C(c)
        norm = rsum @ featc.astype(np.float32)        # [1024]
        recipb = np.ascontiguousarray(
            np.broadcast_to((1.0 / norm)[None, :], (L, NYX)).astype(np.float32))
        zvec = (Gz[:, r] / czsum[r]).astype(np.float32)
        zco = np.tile(zvec, (128, 1))
        zrest = zvec.copy(); zrest[r] = 0.0
        unown = np.ascontiguousarray(blkT[:, r].reshape(128, FB))
        maps.append({
            "zcoo": np.full((128, 1), zvec[r], np.float32),
            "zcor": np.ascontiguousarray(np.tile(zrest, (128, 1))),
            "featr": featr,
            "featc": featc,
            "kyx": kyx_in,
            "unaryt": un,
            "unown": unown,
            "zcoef": np.ascontiguousarray(zco),
            "recipb": recipb,
        })
    return maps


def kernel(image, logits, spatial_ker_weights, bilateral_ker_weights,
           compatibility_matrix):
    from concourse.bass_utils import run_bass_kernel_spmd

    if "nc" not in _CACHE:
        _CACHE["nc"] = _build_nc()
    nc = _CACHE["nc"]

    maps = _host_prep(image, logits)
    ws = np.asarray(spatial_ker_weights, np.float32)
    wb = np.asarray(bilateral_ker_weights, np.float32)
    cm = np.asarray(compatibility_matrix, np.float32)
    w12t = np.zeros((53, L), np.float32)
    w12t[0:L] = (cm @ wb).T
    w12t[32:53] = (cm @ ws).T
    w12t = np.ascontiguousarray(w12t)
    for m in maps:
        m["w12t"] = w12t

    res = run_bass_kernel_spmd(nc, maps, core_ids=list(range(NC)))

    out = np.empty((L, D, H, W), dtype=np.float32)
    for r in range(NC):
        blk = res.results[r]["out"]                   # [128, 168]
        out[:, r] = blk.reshape(128, NT, L).transpose(2, 1, 0).reshape(L, H, W)
    return out[None]

